# revision 48
# baseline (speedup 1.0000x reference)
"""Trainium2 Bass kernel for nn_KnowledgeCircuit (moe_routing).

  h   = einsum('bsd,ndr,bsn->bsr', x, feature_know, feature_know_w)
  out = einsum('bsr,bsn,nrd->bsd', h, restore_know_w, restore_know)

Shapes: B=4, S=2048, D=1024, N=64, R=128.  Data-parallel over the
B*S = 8192 tokens -> 1024 tokens/core on 8 cores; pools replicated.

fp8 DoubleRow scheme (0.5 cyc/output-row vs bf16's 1.0): every main
matmul runs in e4m3 DoubleRow mode where the two K-slots carry a
[hi, residual] decomposition of the *stationary* operand, making it
effectively exact (~0.07% recon err); the moving operand is a single
e4m3 copy fed to both slots through a stride-0 broadcast AP.  The
remaining single-e4m3 error (x in stage 1, g=h*w2c in stage 2) is
halved by mean-shifting the routing weights: w = 0.5 + (w - 0.5); the
0.5-weighted shared term is a cheap matmul against sum_n fk / sum_n rk
(1/64 of the volume), so the fp8 noise is only weighted by (w - 0.5).
Measured end-to-end rel err ~1.7e-2 (gate 2e-2); PE time ~2x better
than the bf16 roofline.

Stage 1: psum[t,512] accumulates [x_hi,x_lo]^T @ FK8 over 8 d-tiles
  (FK8 = e4m3(64*fk)); DVE/Pool stt applies (w1-0.5)/64 into h_v[t,r];
  h_v is seeded with the mean term (x @ 8*fksum)/16 from two fp8-pair
  passes.  h_v -> PE transpose -> hT fp16.
Stage 2: g8[n] = e4m3(hT * (w2-0.5)) on DVE/Pool; psum[d,512]
  accumulates [RK16,RK16r]^T @ [g8,g8] over all pools (RK16 =
  e4m3(16*rk)), seeded with hT @ 8*rksum; drained with a 1/16 scale.
"""

from contextlib import ExitStack

import ml_dtypes
import numpy as np

import concourse.mybir as mybir
import concourse.tile as tile
from concourse import bacc
from concourse.bass_utils import run_bass_kernel_spmd
from concourse.masks import make_identity

F32 = mybir.dt.float32
FP16 = mybir.dt.float16
FP8 = mybir.dt.float8e4
NP_E4 = ml_dtypes.float8_e4m3
DR = mybir.MatmulPerfMode.DoubleRow
MULT = mybir.AluOpType.mult
ADD = mybir.AluOpType.add
COPY = mybir.ActivationFunctionType.Copy

B, S, D, N, R = 4, 2048, 1024, 64, 128
N_CORES = 8
T = B * S // N_CORES  # 1024 tokens per core
TT = T // 128  # 8 token tiles
DK = D // 128  # 8 d tiles
NQ = N // 4  # 16 stage-1 quads
NDP = DK // 2  # 4 d-tile pairs (stage-2 rk layout)


def _bcast2(ap, k, f):
    """[k, f] AP -> [k, 2, f] with a stride-0 middle dim (DR slot dup)."""
    return ap.unsqueeze(1).broadcast_to((k, 2, f))


def build_kernel(debug=False):
    nc = bacc.Bacc(None, target_bir_lowering=False, debug=debug)

    xp_d = nc.dram_tensor("xp", [DK, 128, 2, T], FP8, kind="ExternalInput")
    fkm8_d = nc.dram_tensor("fkm8", [128, DK, 128], FP8, kind="ExternalInput")
    fkmr_d = nc.dram_tensor("fkmr", [128, DK, 128], FP8, kind="ExternalInput")
    fkq_d = nc.dram_tensor("fkq", [NQ, 128, DK, 512], FP8, kind="ExternalInput")
    w1c_d = nc.dram_tensor("w1c", [T, N], F32, kind="ExternalInput")
    w2cT_d = nc.dram_tensor("w2cT", [N, T], FP16, kind="ExternalInput")
    rkp_d = nc.dram_tensor("rkp", [NDP, N, 128, 2, 256], FP8, kind="ExternalInput")
    rks_d = nc.dram_tensor("rks", [128, D], FP16, kind="ExternalInput")
    out_d = nc.dram_tensor("out", [D, T], F32, kind="ExternalOutput")

    with tile.TileContext(nc) as tc, ExitStack() as ctx:
        sb_const = ctx.enter_context(tc.tile_pool(name="const", bufs=1))
        sb_xp = ctx.enter_context(tc.tile_pool(name="xp", bufs=DK))
        sb_fkm = ctx.enter_context(tc.tile_pool(name="fkm", bufs=2))
        sb_fkq = ctx.enter_context(tc.tile_pool(name="fkq", bufs=2))
        sb_w1 = ctx.enter_context(tc.tile_pool(name="w1", bufs=TT))
        sb_hv = ctx.enter_context(tc.tile_pool(name="hv", bufs=TT))
        sb_hT = ctx.enter_context(tc.tile_pool(name="hT", bufs=1))
        sb_hq = ctx.enter_context(tc.tile_pool(name="hq", bufs=4))
        sb_bc = ctx.enter_context(tc.tile_pool(name="bc", bufs=20))
        sb_g = ctx.enter_context(tc.tile_pool(name="g", bufs=N))
        sb_rk = ctx.enter_context(tc.tile_pool(name="rk", bufs=10))
        sb_rks = ctx.enter_context(tc.tile_pool(name="rks", bufs=1))
        sb_ot = ctx.enter_context(tc.tile_pool(name="ot", bufs=8))
        psum = ctx.enter_context(tc.tile_pool(name="ps", bufs=8, space="PSUM"))

        ident = sb_const.tile([128, 128], F32, tag="ident")
        make_identity(nc, ident[:])
        # PE p-state warmup
        for i in range(5):
            wt = psum.tile([128, 128], F32, tag="ps", name=f"warm{i}")
            nc.tensor.transpose(wt[:], ident[:], ident[:])

        # ---- loads: xp split SP/Act so mean-1 can start pipelined; w1t/rks
        # deferred (first needed ~15us / ~130us in) ----
        xp = [sb_xp.tile([128, 2, T], FP8, tag="xp", name=f"xp{i}") for i in range(DK)]
        fkm8 = sb_fkm.tile([128, DK, 128], FP8, tag="fkm", name="fkm8")
        fkmr = sb_fkm.tile([128, DK, 128], FP8, tag="fkm", name="fkmr")
        nc.scalar.dma_start(fkm8[:], fkm8_d[:])
        for dk in range(DK):
            eng_d = nc.sync if dk % 2 == 0 else nc.scalar
            eng_d.dma_start(xp[dk][:], xp_d[dk])
        nc.scalar.dma_start(fkmr[:], fkmr_d[:])
        w1t = [sb_w1.tile([128, N], F32, tag="w1", name=f"w1_{i}") for i in range(TT)]
        rks = sb_rks.tile([128, D], FP16, tag="rks")

        # ---- stage 1 mean term: h_v[tt] = (x @ 8*fksum) / 16 ----
        # dk-outer so the first matmuls only wait on xp[0]; two tt-halves so
        # the first half's PSUM slots recycle before the q-loop needs banks
        h_v = [
            sb_hv.tile([128, R], F32, tag="hv", name=f"hv{i}") for i in range(TT)
        ]
        # fp16 side-accumulators for the even-quad stt (DVE 2x mode needs
        # all-16-bit aps); merged into h_v before the transposes
        h16 = [
            sb_hv.tile([128, R], FP16, tag="h16", name=f"h16_{i}") for i in range(TT)
        ]
        for tt in range(TT):
            nc.gpsimd.memset(h16[tt][:], 0.0)
        for half in (0, 1):
            tts = range(half * 4, half * 4 + 4)
            pms = {
                tt: psum.tile([128, 128], F32, tag="ps", name=f"pm{tt}")
                for tt in tts
            }
            for i, fkm in enumerate((fkm8, fkmr)):
                for dk in range(DK):
                    for tt in tts:
                        nc.tensor.matmul(
                            pms[tt][:],
                            xp[dk][:, :, tt * 128 : (tt + 1) * 128],
                            _bcast2(fkm[:, dk, :], 128, 128),
                            start=(i == 0 and dk == 0),
                            stop=(i == 1 and dk == DK - 1),
                            perf_mode=DR,
                        )
            for tt in tts:
                # DVE is idle at the front; tensor_scalar from PSUM is legal
                nc.vector.tensor_scalar_mul(h_v[tt][:], pms[tt][:], 1.0 / 16.0)

        # ---- stage 1 main ----
        hT = sb_hT.tile([128, T], FP16, tag="hT")
        g8 = [sb_g.tile([128, T], FP8, tag="g", name=f"g{i}") for i in range(N)]
        bcs = {}
        n_emitted = [0]
        g_lo_done = [False]

        def emit_hT(tt):
            tp = psum.tile([128, 128], F32, tag="ps", name=f"tp{tt}")
            nc.tensor.transpose(tp[:], h_v[tt][:], ident[:])
            dst = hT[:, tt * 128 : (tt + 1) * 128]
            if tt % 2 == 0:
                nc.vector.tensor_copy(dst, tp[:])
            else:
                nc.scalar.activation(dst, tp[:], COPY)
            n_emitted[0] += 1

        def _bc(n):
            if n not in bcs:
                bc = sb_bc.tile([128, T], FP16, tag="bc")
                eng_d = nc.sync if n % 2 == 0 else nc.scalar
                eng_d.dma_start(
                    bc[:], w2cT_d[n : n + 1, :].partition_broadcast(128)
                )
                bcs[n] = bc
            return bcs[n]

        def emit_g_half(n, th, eng):
            # g8[n] half = e4m3(hT * (w2-0.5)); SBUF-only so Pool is legal
            sl = slice(th * 512, (th + 1) * 512)
            eng.tensor_mul(g8[n][:, sl], hT[:, sl], _bc(n)[:, sl])

        def emit_g_full(n, eng):
            eng.tensor_mul(g8[n][:], hT[:], _bc(n)[:])

        GWARM = 16  # pools whose lower-half g is produced during stage-1 tail
        fkqs = {}
        for q in range(NQ):
            if q == 0:
                fkqs[0] = sb_fkq.tile([128, DK, 512], FP8, tag="fkq", name="fkq0")
                # dk=0 slice first so q0 can start as soon as mean-1 frees
                # a PSUM slot; remainder follows
                nc.sync.dma_start(fkqs[0][:, 0, :], fkq_d[0, :, 0])
                nc.sync.dma_start(fkqs[0][:, 1:, :], fkq_d[0, :, 1:])
                for tt in range(TT):
                    nc.sync.dma_start(
                        w1t[tt][:], w1c_d[tt * 128 : (tt + 1) * 128, :]
                    )
            if q + 1 < NQ:
                fkqs[q + 1] = sb_fkq.tile(
                    [128, DK, 512], FP8, tag="fkq", name=f"fkq{q + 1}"
                )
                nc.sync.dma_start(fkqs[q + 1][:], fkq_d[q + 1])
            fkq = fkqs.pop(q)
            last_q = q == NQ - 1
            # DVE-stt quads vs Act-weighted+Pool-add quads: front two even
            # quads go to DVE (Act's queue is DMA-heavy at startup)
            dve_q = q % 2 == 1 if q > 3 else q % 2 == 0
            for tt in range(TT):
                if last_q and tt >= 2:
                    # h_v[tt-2]'s stt drains have finished by now
                    emit_hT(tt - 2)
                if last_q and tt == 5 and not g_lo_done[0]:
                    # merges 0-3 + hT copies 0-3 are emitted; Pool starts
                    # the warm lower-half g
                    for n in range(GWARM):
                        emit_g_half(n, 0, nc.gpsimd)
                    g_lo_done[0] = True

                hp = psum.tile([128, 512], F32, tag="ps", name=f"hp{q}_{tt}")
                for dk in range(DK):
                    nc.tensor.matmul(
                        hp[:],
                        xp[dk][:, :, tt * 128 : (tt + 1) * 128],
                        _bcast2(fkq[:, dk, :], 128, 512),
                        start=(dk == 0),
                        stop=(dk == DK - 1),
                        perf_mode=DR,
                    )
                # stt is DVE-only on real HW (TensorScalarPtr is not in the
                # Pool ISA, and Pool can't read PSUM anyway). Three drain
                # paths balance the engines: odd quads DVE-stt from PSUM;
                # most even quads Act-copy (fp16) + DVE-stt from SBUF; two
                # quads Act-weighted-copy (scale=w1 column) + Pool adds.
                if dve_q:
                    for i in range(4):
                        n = q * 4 + i
                        nc.vector.scalar_tensor_tensor(
                            h_v[tt][:],
                            hp[:, i * 128 : (i + 1) * 128],
                            w1t[tt][:, n : n + 1],
                            h_v[tt][:],
                            MULT,
                            ADD,
                        )
                else:
                    for i in range(4):
                        n = q * 4 + i
                        wv = sb_hq.tile([128, 128], FP16, tag="wv", bufs=8)
                        nc.scalar.activation(
                            wv[:],
                            hp[:, i * 128 : (i + 1) * 128],
                            COPY,
                            scale=w1t[tt][:, n : n + 1],
                        )
                        nc.gpsimd.tensor_tensor(
                            h16[tt][:], h16[tt][:], wv[:], ADD
                        )
                if last_q:
                    # merge the fp16 side-accumulator as soon as this token
                    # tile's final stt lands; upper-half merges on DVE so
                    # Pool's queue is free for the warm g production
                    eng_m = nc.gpsimd if tt < 4 else nc.vector
                    eng_m.tensor_tensor(h_v[tt][:], h_v[tt][:], h16[tt][:], ADD)
        while n_emitted[0] < TT:
            emit_hT(n_emitted[0])
        # rksum load just before first use (Act)
        nc.scalar.dma_start(rks[:], rks_d[:])

        # ---- stage 2: out[d, t] accumulation in d-block passes ----
        passes = [(0, 2), (2, 4), (4, 6), (6, 8)]

        def drain(po_ap, d_lo, d_hi, t_lo, t_hi, i):
            ot = sb_ot.tile([128, t_hi - t_lo], F32, tag="ot")
            if i % 2 == 0:
                nc.vector.tensor_scalar_mul(ot[:], po_ap, 1.0 / 16.0)
            else:
                nc.scalar.activation(ot[:], po_ap, COPY, scale=1.0 / 16.0)
            # out DMAs on Act: SP stays free for rkt streaming
            nc.scalar.dma_start(out_d[d_lo:d_hi, t_lo:t_hi], ot[:])

        for pq, (d0, d1) in enumerate(passes):
            dp = d0 // 2
            rkt = []
            for nb in range(N // 8):
                t_ = sb_rk.tile([128, 8, 2, 256], FP8, tag="rk", name=f"rk{pq}_{nb}")
                nc.sync.dma_start(
                    t_[:],
                    rkp_d[dp, nb * 8 : (nb + 1) * 8].rearrange("n p s e -> p n s e"),
                )
                rkt.append(t_)

            def rk_st(n, j):
                return rkt[n // 8][:, n % 8, :, j * 128 : (j + 1) * 128]

            po = [
                psum.tile([128, 512], F32, tag="ps", name=f"po{pq}_{i}")
                for i in range(4)
            ]

            def mean2(j, th):
                # 16*(0.5*h@rksum) = hT @ (8*rksum); th=0 only needs hT 0:512
                nc.tensor.matmul(
                    po[j * 2 + th][:],
                    rks[:, (d0 + j) * 128 : (d0 + j + 1) * 128],
                    hT[:, th * 512 : (th + 1) * 512],
                    start=True,
                    stop=False,
                )

            def mm(n, j, th, stop):
                nc.tensor.matmul(
                    po[j * 2 + th][:],
                    rk_st(n, j),
                    _bcast2(g8[n][:, th * 512 : (th + 1) * 512], 128, 512),
                    start=False,
                    stop=stop,
                    perf_mode=DR,
                )

            if pq == 0:
                # warm entry: lower-half g for the first GWARM pools was
                # produced during the stage-1 tail; th=0 matmuls consume
                # them while the upper halves and full-width rest stream
                mean2(0, 0)
                mean2(1, 0)
                for n in range(GWARM):
                    mm(n, 0, 0, False)
                    mm(n, 1, 0, False)
                for n in range(GWARM):
                    emit_g_half(n, 1, nc.vector if n % 2 == 0 else nc.gpsimd)
                for n in range(GWARM, N):
                    # full-width single op (cheaper than two halves)
                    emit_g_full(n, nc.vector if n % 2 == 0 else nc.gpsimd)
                mean2(0, 1)
                mean2(1, 1)
                for n in range(GWARM):
                    mm(n, 0, 1, False)
                    mm(n, 1, 1, False)
                for n in range(GWARM, N):
                    for j in range(2):
                        for th in range(2):
                            mm(n, j, th, n == N - 1)
                for i in range(4):
                    j, th = divmod(i, 2)
                    drain(po[i][:], (d0 + j) * 128, (d0 + j + 1) * 128,
                          th * 512, (th + 1) * 512, i)
            elif pq < len(passes) - 1:
                for j in range(2):
                    for th in range(2):
                        mean2(j, th)
                for n in range(N):
                    for j in range(2):
                        for th in range(2):
                            mm(n, j, th, n == N - 1)
                for i in range(4):
                    j, th = divmod(i, 2)
                    drain(po[i][:], (d0 + j) * 128, (d0 + j + 1) * 128,
                          th * 512, (th + 1) * 512, i)
            else:
                # final pass: stagger banks LAG pools apart so drains overlap
                # the later banks' matmuls; the last bank is split in half so
                # only a [128,256] drain extends the kernel tail
                LAG = 6
                po3a = psum.tile([128, 256], F32, tag="ps", name="po3a")
                po3b = psum.tile([128, 256], F32, tag="ps", name="po3b")
                # windows: (bank, j, t_lo, t_hi)
                wins = [
                    (po[0][:], 0, 0, 512),
                    (po[1][:], 0, 512, 1024),
                    (po[2][:], 1, 0, 512),
                    (po3a[:], 1, 512, 768),
                    (po3b[:], 1, 768, 1024),
                ]
                for ap, j, t_lo, t_hi in wins:
                    nc.tensor.matmul(ap, rks[:, (d0 + j) * 128 : (d0 + j + 1) * 128],
                                     hT[:, t_lo:t_hi], start=True, stop=False)
                for jj in range(N + (len(wins) - 1) * LAG):
                    for b, (ap, j, t_lo, t_hi) in enumerate(wins):
                        m = jj - b * LAG
                        if not 0 <= m < N:
                            continue
                        nc.tensor.matmul(
                            ap,
                            rk_st(m, j),
                            _bcast2(g8[m][:, t_lo:t_hi], 128, t_hi - t_lo),
                            start=False,
                            stop=(m == N - 1),
                            perf_mode=DR,
                        )
                        if m == N - 1:
                            drain(ap, (d0 + j) * 128, (d0 + j + 1) * 128,
                                  t_lo, t_hi, b)

    nc.compile()
    return nc


_NC_CACHE = {}


def _get_nc():
    if "nc" not in _NC_CACHE:
        _NC_CACHE["nc"] = build_kernel(debug=False)
    return _NC_CACHE["nc"]


def _q8(a):
    return np.asarray(a, dtype=np.float32).astype(NP_E4)


def _shard_inputs(x, feature_know_w, restore_know_w, feature_know, restore_know):
    fk = np.asarray(feature_know, dtype=np.float32)  # [N, D, R]
    rk = np.asarray(restore_know, dtype=np.float32)  # [N, R, D]

    # FK8 moving layout [q, p, dk, i*128+r]
    FK8 = _q8(fk * 64.0)
    fkq = np.ascontiguousarray(
        FK8.reshape(NQ, 4, DK, 128, R).transpose(0, 3, 2, 1, 4).reshape(NQ, 128, DK, 4 * R)
    )
    fksum8 = fk.sum(0) * 8.0  # [D, R]
    FKS8 = _q8(fksum8)
    FKSr = _q8(fksum8 - FKS8.astype(np.float32))
    fkm8 = np.ascontiguousarray(FKS8.reshape(DK, 128, R).transpose(1, 0, 2))
    fkmr = np.ascontiguousarray(FKSr.reshape(DK, 128, R).transpose(1, 0, 2))

    RK16 = _q8(rk * 16.0)
    RK16r = _q8(rk * 16.0 - RK16.astype(np.float32))
    # rkp [dp, n, r, s, j*128+dd]
    pair = np.stack([RK16, RK16r], axis=0)  # [s, n, r, D]
    rkp = np.ascontiguousarray(
        pair.reshape(2, N, R, NDP, 2, 128).transpose(3, 1, 2, 0, 4, 5).reshape(NDP, N, R, 2, 256)
    )
    rks = (rk.sum(0) * 8.0).astype(np.float16)  # [R, D]

    xc = np.asarray(x, dtype=np.float32).reshape(N_CORES, T, D)
    w1 = np.asarray(feature_know_w, dtype=np.float32).reshape(N_CORES, T, N)
    w2 = np.asarray(restore_know_w, dtype=np.float32).reshape(N_CORES, T, N)

    in_maps = []
    for c in range(N_CORES):
        xT = np.ascontiguousarray(xc[c].T)  # [D, T]
        x_hi = xT.astype(NP_E4)
        x_lo = (xT - x_hi.astype(np.float32)).astype(NP_E4)
        xpair = np.stack([x_hi, x_lo], axis=1)  # [D, 2, T]
        xp = np.ascontiguousarray(xpair.reshape(DK, 128, 2, T))
        w1c = np.ascontiguousarray((w1[c] - 0.5) / 64.0)
        w2cT = np.ascontiguousarray((w2[c] - 0.5).T.astype(np.float16))
        in_maps.append(
            {
                "xp": xp,
                "fkm8": fkm8,
                "fkmr": fkmr,
                "fkq": fkq,
                "w1c": w1c,
                "w2cT": w2cT,
                "rkp": rkp,
                "rks": rks,
            }
        )
    return in_maps


def _unshard_out(per_core_outs):
    stacked = np.stack(per_core_outs, axis=0)  # [C, D, T]
    return np.ascontiguousarray(stacked.transpose(0, 2, 1)).reshape(B, S, D)


def run(in_maps, **kwargs):
    nc = _get_nc()
    return run_bass_kernel_spmd(nc, in_maps, core_ids=list(range(N_CORES)), **kwargs)


def kernel(x, feature_know_w, restore_know_w, feature_know, restore_know, **_):
    in_maps = _shard_inputs(
        x, feature_know_w, restore_know_w, feature_know, restore_know
    )
    res = run(in_maps)
    return _unshard_out([r["out"] for r in res.results])


# revision 68
# speedup vs baseline: 1.0204x; 1.0204x over previous
"""Trainium2 Bass kernel for nn_KnowledgeCircuit (moe_routing).

  h   = einsum('bsd,ndr,bsn->bsr', x, feature_know, feature_know_w)
  out = einsum('bsr,bsn,nrd->bsd', h, restore_know_w, restore_know)

Shapes: B=4, S=2048, D=1024, N=64, R=128.  Data-parallel over the
B*S = 8192 tokens -> 1024 tokens/core on 8 cores; pools replicated.

fp8 DoubleRow scheme (0.5 cyc/output-row vs bf16's 1.0): every main
matmul runs in e4m3 DoubleRow mode where the two K-slots carry a
[hi, residual] decomposition of the *stationary* operand, making it
effectively exact (~0.07% recon err); the moving operand is a single
e4m3 copy fed to both slots through a stride-0 broadcast AP.  The
remaining single-e4m3 error (x in stage 1, g=h*w2c in stage 2) is
halved by mean-shifting the routing weights: w = 0.5 + (w - 0.5); the
0.5-weighted shared term is a cheap matmul against sum_n fk / sum_n rk
(1/64 of the volume), so the fp8 noise is only weighted by (w - 0.5).
Measured end-to-end rel err ~1.7e-2 (gate 2e-2); PE time ~2x better
than the bf16 roofline.

Stage 1: psum[t,512] accumulates [x_hi,x_lo]^T @ FK8 over 8 d-tiles
  (FK8 = e4m3(64*fk)); DVE/Pool stt applies (w1-0.5)/64 into h_v[t,r];
  h_v is seeded with the mean term (x @ 8*fksum)/16 from two fp8-pair
  passes.  h_v -> PE transpose -> hT fp16.
Stage 2: g8[n] = e4m3(hT * (w2-0.5)) on DVE/Pool; psum[d,512]
  accumulates [RK16,RK16r]^T @ [g8,g8] over all pools (RK16 =
  e4m3(16*rk)), seeded with hT @ 8*rksum; drained with a 1/16 scale.
"""

from contextlib import ExitStack

import ml_dtypes
import numpy as np

import concourse.mybir as mybir
import concourse.tile as tile
from concourse import bacc
from concourse.bass_utils import run_bass_kernel_spmd
from concourse.masks import make_identity

F32 = mybir.dt.float32
FP16 = mybir.dt.float16
FP8 = mybir.dt.float8e4
NP_E4 = ml_dtypes.float8_e4m3
DR = mybir.MatmulPerfMode.DoubleRow
MULT = mybir.AluOpType.mult
ADD = mybir.AluOpType.add
COPY = mybir.ActivationFunctionType.Copy

B, S, D, N, R = 4, 2048, 1024, 64, 128
N_CORES = 8
T = B * S // N_CORES  # 1024 tokens per core
TT = T // 128  # 8 token tiles
DK = D // 128  # 8 d tiles
NQ = N // 4  # 16 stage-1 quads
NDP = DK // 2  # 4 d-tile pairs (stage-2 rk layout)


def _bcast2(ap, k, f):
    """[k, f] AP -> [k, 2, f] with a stride-0 middle dim (DR slot dup)."""
    return ap.unsqueeze(1).broadcast_to((k, 2, f))


def build_kernel(debug=False):
    nc = bacc.Bacc(None, target_bir_lowering=False, debug=debug)

    xp_d = nc.dram_tensor("xp", [DK, 128, 2, T], FP8, kind="ExternalInput")
    fkm8_d = nc.dram_tensor("fkm8", [128, DK, 128], FP8, kind="ExternalInput")
    fkmr_d = nc.dram_tensor("fkmr", [128, DK, 128], FP8, kind="ExternalInput")
    fkq_d = nc.dram_tensor("fkq", [NQ, 128, DK, 512], FP8, kind="ExternalInput")
    w1c_d = nc.dram_tensor("w1c", [T, N], F32, kind="ExternalInput")
    w2cT_d = nc.dram_tensor("w2cT", [N, T], FP16, kind="ExternalInput")
    rkp_d = nc.dram_tensor("rkp", [NDP, N, 128, 2, 256], FP8, kind="ExternalInput")
    rks_d = nc.dram_tensor("rks", [128, D], FP16, kind="ExternalInput")
    out_d = nc.dram_tensor("out", [D, T], F32, kind="ExternalOutput")

    with tile.TileContext(nc) as tc, ExitStack() as ctx:
        sb_const = ctx.enter_context(tc.tile_pool(name="const", bufs=1))
        sb_xp = ctx.enter_context(tc.tile_pool(name="xp", bufs=DK))
        sb_fkm = ctx.enter_context(tc.tile_pool(name="fkm", bufs=2))
        sb_fkq = ctx.enter_context(tc.tile_pool(name="fkq", bufs=2))
        sb_w1 = ctx.enter_context(tc.tile_pool(name="w1", bufs=TT))
        sb_hv = ctx.enter_context(tc.tile_pool(name="hv", bufs=TT))
        sb_hT = ctx.enter_context(tc.tile_pool(name="hT", bufs=1))
        sb_hq = ctx.enter_context(tc.tile_pool(name="hq", bufs=4))
        sb_bc = ctx.enter_context(tc.tile_pool(name="bc", bufs=20))
        sb_g = ctx.enter_context(tc.tile_pool(name="g", bufs=N))
        sb_rk = ctx.enter_context(tc.tile_pool(name="rk", bufs=10))
        sb_rks = ctx.enter_context(tc.tile_pool(name="rks", bufs=1))
        sb_ot = ctx.enter_context(tc.tile_pool(name="ot", bufs=8))
        psum = ctx.enter_context(tc.tile_pool(name="ps", bufs=8, space="PSUM"))

        ident = sb_const.tile([128, 128], F32, tag="ident")
        make_identity(nc, ident[:])
        # PE p-state warmup: enough dummies to keep the clock ramping while
        # the first xp/fkm DMAs land, so mean-1 runs at full speed
        for i in range(8):
            wt = psum.tile([128, 128], F32, tag="ps", name=f"warm{i}")
            nc.tensor.transpose(wt[:], ident[:], ident[:])

        # ---- loads: xp split SP/Act so mean-1 can start pipelined; w1t/rks
        # deferred (first needed ~15us / ~130us in) ----
        xp = [sb_xp.tile([128, 2, T], FP8, tag="xp", name=f"xp{i}") for i in range(DK)]

        def xp_sl(dk, t0, t1):
            return xp[dk][:, :, t0:t1]

        fkm8 = sb_fkm.tile([128, DK, 128], FP8, tag="fkm", name="fkm8")
        fkmr = sb_fkm.tile([128, DK, 128], FP8, tag="fkm", name="fkmr")
        nc.scalar.dma_start(fkm8[:], fkm8_d[:])
        for dk in range(DK):
            eng_d = nc.sync if dk % 2 == 0 else nc.scalar
            eng_d.dma_start(xp[dk][:], xp_d[dk])
        nc.scalar.dma_start(fkmr[:], fkmr_d[:])
        w1t = [sb_w1.tile([128, N], F32, tag="w1", name=f"w1_{i}") for i in range(TT)]
        rks = sb_rks.tile([128, D], FP16, tag="rks")

        # ---- stage 1 mean term: h_v[tt] = (x @ 8*fksum) / 16 ----
        # dk-outer so the first matmuls only wait on xp[0]; two tt-halves so
        # the first half's PSUM slots recycle before the q-loop needs banks
        h_v = [
            sb_hv.tile([128, R], F32, tag="hv", name=f"hv{i}") for i in range(TT)
        ]
        # fp16 side-accumulators for the even-quad stt (DVE 2x mode needs
        # all-16-bit aps); merged into h_v before the transposes
        h16 = [
            sb_hv.tile([128, R], FP16, tag="h16", name=f"h16_{i}") for i in range(TT)
        ]
        for tt in range(TT):
            nc.gpsimd.memset(h16[tt][:], 0.0)
        for half in (0, 1):
            tts = range(half * 4, half * 4 + 4)
            pms = {
                tt: psum.tile([128, 128], F32, tag="ps", name=f"pm{tt}")
                for tt in tts
            }
            for i, fkm in enumerate((fkm8, fkmr)):
                for dk in range(DK):
                    for tt in tts:
                        nc.tensor.matmul(
                            pms[tt][:],
                            xp_sl(dk, tt * 128, (tt + 1) * 128),
                            _bcast2(fkm[:, dk, :], 128, 128),
                            start=(i == 0 and dk == 0),
                            stop=(i == 1 and dk == DK - 1),
                            perf_mode=DR,
                        )
            for tt in tts:
                # DVE is idle at the front; tensor_scalar from PSUM is legal
                nc.vector.tensor_scalar_mul(h_v[tt][:], pms[tt][:], 1.0 / 16.0)

        # ---- stage 1 main ----
        hT = sb_hT.tile([128, T], FP16, tag="hT")
        g8 = [sb_g.tile([128, T], FP8, tag="g", name=f"g{i}") for i in range(N)]
        bcs = {}
        n_emitted = [0]
        g_lo_done = [False]

        def emit_hT(tt):
            tp = psum.tile([128, 128], F32, tag="ps", name=f"tp{tt}")
            nc.tensor.transpose(tp[:], h_v[tt][:], ident[:])
            dst = hT[:, tt * 128 : (tt + 1) * 128]
            if tt % 2 == 0:
                nc.vector.tensor_copy(dst, tp[:])
            else:
                nc.scalar.activation(dst, tp[:], COPY)
            n_emitted[0] += 1

        def _bc(n):
            if n not in bcs:
                bc = sb_bc.tile([128, T], FP16, tag="bc")
                eng_d = nc.sync if n % 2 == 0 else nc.scalar
                eng_d.dma_start(
                    bc[:], w2cT_d[n : n + 1, :].partition_broadcast(128)
                )
                bcs[n] = bc
            return bcs[n]

        def emit_g_half(n, th, eng):
            # g8[n] half = e4m3(hT * (w2-0.5)); SBUF-only so Pool is legal
            sl = slice(th * 512, (th + 1) * 512)
            eng.tensor_mul(g8[n][:, sl], hT[:, sl], _bc(n)[:, sl])

        def emit_g_full(n, eng):
            eng.tensor_mul(g8[n][:], hT[:], _bc(n)[:])

        GWARM = 16  # pools whose lower-half g is produced during stage-1 tail
        fkqs = {}
        for q in range(NQ):
            if q == 0:
                fkqs[0] = sb_fkq.tile([128, DK, 512], FP8, tag="fkq", name="fkq0")
                # dk=0 slice first so q0 can start as soon as mean-1 frees
                # a PSUM slot; remainder follows
                nc.sync.dma_start(fkqs[0][:, 0, :], fkq_d[0, :, 0])
                nc.sync.dma_start(fkqs[0][:, 1:, :], fkq_d[0, :, 1:])
                for tt in range(TT):
                    nc.sync.dma_start(
                        w1t[tt][:], w1c_d[tt * 128 : (tt + 1) * 128, :]
                    )
            if q + 1 < NQ:
                fkqs[q + 1] = sb_fkq.tile(
                    [128, DK, 512], FP8, tag="fkq", name=f"fkq{q + 1}"
                )
                nc.sync.dma_start(fkqs[q + 1][:], fkq_d[q + 1])
            fkq = fkqs.pop(q)
            last_q = q == NQ - 1
            if q == 13:
                # pre-issue the warm pools' w2c broadcasts: no deps, and the
                # tiles must be resident before the warm g muls at q15
                for n in range(GWARM):
                    _bc(n)
            # DVE-stt quads vs Act-weighted+Pool-add quads: front even quads
            # go to DVE (Act's queue is DMA-heavy at startup); parity flips
            # at q7 so the only same-engine adjacency lands mid-kernel
            dve_q = q % 2 == 0 if q <= 6 else q % 2 == 1
            if last_q:
                # h16 is final after q14 (the last Act+Pool quad): fold it
                # into h_v up front so each transpose waits only on q15's
                # own stt chain
                for tt in range(TT):
                    # all on Pool: they chain naturally after q14's adds
                    nc.gpsimd.tensor_tensor(h_v[tt][:], h_v[tt][:], h16[tt][:], ADD)
            for tt in range(TT):
                if last_q and tt == 6 and not g_lo_done[0]:
                    # merges 0-3 + hT copies 0-3 are emitted; Pool starts
                    # the warm lower-half g
                    for n in range(GWARM):
                        emit_g_half(n, 0, nc.gpsimd)
                    g_lo_done[0] = True

                hp = psum.tile([128, 512], F32, tag="ps", name=f"hp{q}_{tt}")
                for dk in range(DK):
                    nc.tensor.matmul(
                        hp[:],
                        xp_sl(dk, tt * 128, (tt + 1) * 128),
                        _bcast2(fkq[:, dk, :], 128, 512),
                        start=(dk == 0),
                        stop=(dk == DK - 1),
                        perf_mode=DR,
                    )
                # stt is DVE-only on real HW (TensorScalarPtr is not in the
                # Pool ISA, and Pool can't read PSUM anyway). Three drain
                # paths balance the engines: odd quads DVE-stt from PSUM;
                # most even quads Act-copy (fp16) + DVE-stt from SBUF; two
                # quads Act-weighted-copy (scale=w1 column) + Pool adds.
                if dve_q:
                    for i in range(4):
                        n = q * 4 + i
                        nc.vector.scalar_tensor_tensor(
                            h_v[tt][:],
                            hp[:, i * 128 : (i + 1) * 128],
                            w1t[tt][:, n : n + 1],
                            h_v[tt][:],
                            MULT,
                            ADD,
                        )
                else:
                    for i in range(4):
                        n = q * 4 + i
                        wv = sb_hq.tile([128, 128], FP16, tag="wv", bufs=8)
                        nc.scalar.activation(
                            wv[:],
                            hp[:, i * 128 : (i + 1) * 128],
                            COPY,
                            scale=w1t[tt][:, n : n + 1],
                        )
                        nc.gpsimd.tensor_tensor(
                            h16[tt][:], h16[tt][:], wv[:], ADD
                        )
                if last_q and tt >= 2:
                    # transpose for tt-2: its stt chain has had two full
                    # iterations to drain
                    emit_hT(tt - 2)
        while n_emitted[0] < TT - 1:
            emit_hT(n_emitted[0])
        # rksum load just before first use (Act)
        nc.scalar.dma_start(rks[:], rks_d[:])

        # ---- stage 2: out[d, t] accumulation in d-block passes ----
        passes = [(0, 2), (2, 4), (4, 6), (6, 8)]

        def drain(po_ap, d_lo, d_hi, t_lo, t_hi, i, dma_eng=None):
            ot = sb_ot.tile([128, t_hi - t_lo], F32, tag="ot")
            if i % 2 == 0:
                nc.vector.tensor_scalar_mul(ot[:], po_ap, 1.0 / 16.0)
            else:
                nc.scalar.activation(ot[:], po_ap, COPY, scale=1.0 / 16.0)
            # out DMAs on Act: SP stays free for rkt streaming
            (dma_eng or nc.scalar).dma_start(out_d[d_lo:d_hi, t_lo:t_hi], ot[:])

        for pq, (d0, d1) in enumerate(passes):
            dp = d0 // 2
            rkt = []
            for nb in range(N // 8):
                t_ = sb_rk.tile([128, 8, 2, 256], FP8, tag="rk", name=f"rk{pq}_{nb}")
                nc.sync.dma_start(
                    t_[:],
                    rkp_d[dp, nb * 8 : (nb + 1) * 8].rearrange("n p s e -> p n s e"),
                )
                rkt.append(t_)

            def rk_st(n, j):
                return rkt[n // 8][:, n % 8, :, j * 128 : (j + 1) * 128]

            po = [
                psum.tile([128, 512], F32, tag="ps", name=f"po{pq}_{i}")
                for i in range(4)
            ]

            def mean2(j, th):
                # 16*(0.5*h@rksum) = hT @ (8*rksum); th=0 only needs hT 0:512
                nc.tensor.matmul(
                    po[j * 2 + th][:],
                    rks[:, (d0 + j) * 128 : (d0 + j + 1) * 128],
                    hT[:, th * 512 : (th + 1) * 512],
                    start=True,
                    stop=False,
                )

            def mm(n, j, th, stop):
                nc.tensor.matmul(
                    po[j * 2 + th][:],
                    rk_st(n, j),
                    _bcast2(g8[n][:, th * 512 : (th + 1) * 512], 128, 512),
                    start=False,
                    stop=stop,
                    perf_mode=DR,
                )

            if pq == 0:
                # warm entry: lower-half g for the first GWARM pools was
                # produced during the stage-1 tail; th=0 matmuls consume
                # them while the upper halves and full-width rest stream
                mean2(0, 0)
                mean2(1, 0)
                for n in range(GWARM):
                    mm(n, 0, 0, False)
                    mm(n, 1, 0, False)
                # tt7's transpose lands here, in the shadow of the warm mms
                emit_hT(TT - 1)
                for n in range(GWARM):
                    emit_g_half(n, 1, nc.vector if n % 2 == 0 else nc.gpsimd)
                for n in range(GWARM, N):
                    # full-width single op (cheaper than two halves)
                    emit_g_full(n, nc.vector if n % 2 == 0 else nc.gpsimd)
                mean2(0, 1)
                mean2(1, 1)
                for n in range(GWARM):
                    mm(n, 0, 1, False)
                    mm(n, 1, 1, False)
                for n in range(GWARM, N):
                    for j in range(2):
                        for th in range(2):
                            mm(n, j, th, n == N - 1)
                for i in range(4):
                    j, th = divmod(i, 2)
                    drain(po[i][:], (d0 + j) * 128, (d0 + j + 1) * 128,
                          th * 512, (th + 1) * 512, i)
            elif pq < len(passes) - 1:
                for j in range(2):
                    for th in range(2):
                        mean2(j, th)
                for n in range(N):
                    for j in range(2):
                        for th in range(2):
                            mm(n, j, th, n == N - 1)
                for i in range(4):
                    j, th = divmod(i, 2)
                    drain(po[i][:], (d0 + j) * 128, (d0 + j + 1) * 128,
                          th * 512, (th + 1) * 512, i)
            else:
                # final pass: stagger banks LAG pools apart so drains overlap
                # the later banks' matmuls; the last bank is split in half so
                # only a [128,256] drain extends the kernel tail
                LAG = 6
                po3a = psum.tile([128, 256], F32, tag="ps", name="po3a")
                po3b = psum.tile([128, 256], F32, tag="ps", name="po3b")
                # windows: (bank, j, t_lo, t_hi)
                wins = [
                    (po[0][:], 0, 0, 512),
                    (po[1][:], 0, 512, 1024),
                    (po[2][:], 1, 0, 512),
                    (po3a[:], 1, 512, 768),
                    (po3b[:], 1, 768, 1024),
                ]
                for ap, j, t_lo, t_hi in wins:
                    nc.tensor.matmul(ap, rks[:, (d0 + j) * 128 : (d0 + j + 1) * 128],
                                     hT[:, t_lo:t_hi], start=True, stop=False)
                for jj in range(N + (len(wins) - 1) * LAG):
                    for b, (ap, j, t_lo, t_hi) in enumerate(wins):
                        m = jj - b * LAG
                        if not 0 <= m < N:
                            continue
                        nc.tensor.matmul(
                            ap,
                            rk_st(m, j),
                            _bcast2(g8[m][:, t_lo:t_hi], 128, t_hi - t_lo),
                            start=False,
                            stop=(m == N - 1),
                            perf_mode=DR,
                        )
                        if m == N - 1:
                            drain(ap, (d0 + j) * 128, (d0 + j + 1) * 128,
                                  t_lo, t_hi, b,
                                  dma_eng=nc.sync if b >= 3 else None)

    nc.compile()
    return nc


_NC_CACHE = {}


def _get_nc():
    if "nc" not in _NC_CACHE:
        _NC_CACHE["nc"] = build_kernel(debug=False)
    return _NC_CACHE["nc"]


def _q8(a):
    return np.asarray(a, dtype=np.float32).astype(NP_E4)


def _shard_inputs(x, feature_know_w, restore_know_w, feature_know, restore_know):
    fk = np.asarray(feature_know, dtype=np.float32)  # [N, D, R]
    rk = np.asarray(restore_know, dtype=np.float32)  # [N, R, D]

    # FK8 moving layout [q, p, dk, i*128+r]
    FK8 = _q8(fk * 64.0)
    fkq = np.ascontiguousarray(
        FK8.reshape(NQ, 4, DK, 128, R).transpose(0, 3, 2, 1, 4).reshape(NQ, 128, DK, 4 * R)
    )
    fksum8 = fk.sum(0) * 8.0  # [D, R]
    FKS8 = _q8(fksum8)
    FKSr = _q8(fksum8 - FKS8.astype(np.float32))
    fkm8 = np.ascontiguousarray(FKS8.reshape(DK, 128, R).transpose(1, 0, 2))
    fkmr = np.ascontiguousarray(FKSr.reshape(DK, 128, R).transpose(1, 0, 2))

    RK16 = _q8(rk * 16.0)
    RK16r = _q8(rk * 16.0 - RK16.astype(np.float32))
    # rkp [dp, n, r, s, j*128+dd]
    pair = np.stack([RK16, RK16r], axis=0)  # [s, n, r, D]
    rkp = np.ascontiguousarray(
        pair.reshape(2, N, R, NDP, 2, 128).transpose(3, 1, 2, 0, 4, 5).reshape(NDP, N, R, 2, 256)
    )
    rks = (rk.sum(0) * 8.0).astype(np.float16)  # [R, D]

    xc = np.asarray(x, dtype=np.float32).reshape(N_CORES, T, D)
    w1 = np.asarray(feature_know_w, dtype=np.float32).reshape(N_CORES, T, N)
    w2 = np.asarray(restore_know_w, dtype=np.float32).reshape(N_CORES, T, N)

    in_maps = []
    for c in range(N_CORES):
        xT = np.ascontiguousarray(xc[c].T)  # [D, T]
        x_hi = xT.astype(NP_E4)
        x_lo = (xT - x_hi.astype(np.float32)).astype(NP_E4)
        xpair = np.stack([x_hi, x_lo], axis=1)  # [D, 2, T]
        xp = np.ascontiguousarray(xpair.reshape(DK, 128, 2, T))
        w1c = np.ascontiguousarray((w1[c] - 0.5) / 64.0)
        w2cT = np.ascontiguousarray((w2[c] - 0.5).T.astype(np.float16))
        in_maps.append(
            {
                "xp": xp,
                "fkm8": fkm8,
                "fkmr": fkmr,
                "fkq": fkq,
                "w1c": w1c,
                "w2cT": w2cT,
                "rkp": rkp,
                "rks": rks,
            }
        )
    return in_maps


def _unshard_out(per_core_outs):
    stacked = np.stack(per_core_outs, axis=0)  # [C, D, T]
    return np.ascontiguousarray(stacked.transpose(0, 2, 1)).reshape(B, S, D)


def run(in_maps, **kwargs):
    nc = _get_nc()
    return run_bass_kernel_spmd(nc, in_maps, core_ids=list(range(N_CORES)), **kwargs)


def kernel(x, feature_know_w, restore_know_w, feature_know, restore_know, **_):
    in_maps = _shard_inputs(
        x, feature_know_w, restore_know_w, feature_know, restore_know
    )
    res = run(in_maps)
    return _unshard_out([r["out"] for r in res.results])
